# revision 1
# baseline (speedup 1.0000x reference)
"""Self-contained TRN2 Bass kernel v3: single-head encoder self-attention.

out = softmax((X Wq / sqrt(128)) (X Wk)^T, axis=keys) @ (X Wv)
(The reference's query-axis mask is a softmax no-op; masks and
encoder_output_embedding are unused.)

Sharding: core c handles batch c//4, query rows (c%4)*1024..+1024; K/V for
the full 4096-token batch sequence are recomputed per core. Per-core x is
rolled so the core's query rows are rows 0..1023 (softmax/PV are
key-permutation invariant).

Per-core v3 pipeline (software-pipelined two-pass softmax):
  setup: slab DMA -> PE transpose -> X^T; K^T/Q^T/V^T projections (fp32r,
         fixed stationary); V^T cast fp16 -> one DMA-XBAR transpose -> V
         tiles.  X^T copies on DVE, K/Q on ACT (parallel streams).
  prepass est(s): 8 fp32r S chunks [128,512] (2 single-bank PSUM tiles)
         -> DVE negated chunk maxes -> negm(s).  Runs ONE BLOCK AHEAD of
         the main pass so negm never gates the exp stream.
  main(s): 8 fp32r S chunks into [128,1024] 2-bank PSUM quarters (2 tiles)
         -> ACT Exp(bias=negm(s), accum_out -> l4all[:,s,q]) -> P_s fp16
         -> one DMA-XBAR transpose -> P^T tiles pT[d, j, s, q].
  PV in query-halves (stationary = V tile j, moving = pT[:, j, g*4:g*4+4, :]
         512 cols, PSUM [128,512] 1 bank), half 0 overlapped under blocks
         5-7; per half: ACT copy -> PE transpose back -> DVE 1/l scale
         (l8 from one deferred reduce of l4all) -> DMA out.
"""
import contextlib

import numpy as np

import concourse.tile as tile
from concourse import bacc, mybir
from concourse.bass_utils import run_bass_kernel_spmd

F32 = mybir.dt.float32
F32R = mybir.dt.float32r
F16 = mybir.dt.float16
AX = mybir.AxisListType
ALU = mybir.AluOpType
ACTF = mybir.ActivationFunctionType

D = 128
B_SZ = 2
S_SRC = 4096
N_CORES = 8
N_ROWS = (B_SZ * S_SRC) // N_CORES  # 1024 query rows per core

_NC_CACHE = {}


def _make_identity(nc, ap):
    nc.gpsimd.memset(ap, 0.0)
    nc.gpsimd.affine_select(
        out=ap, in_=ap, compare_op=ALU.not_equal, fill=1.0, base=0,
        pattern=[[-1, ap.shape[1]]], channel_multiplier=1)


def _build_attn(n_tok=S_SRC, n_rows=N_ROWS, n_cores=N_CORES, m_repeat=None):
    nb = n_rows // 128          # 128-row query blocks
    tch = n_tok // 128          # 128-key chunks
    kch = n_tok // 512          # 512-key chunks
    nqt = 4                     # exp quarters per block
    qw = n_tok // nqt

    nc = bacc.Bacc("TRN2", target_bir_lowering=False, debug=False,
                   num_devices=n_cores)
    x_d = nc.dram_tensor("x", [n_tok, D], F32, kind="ExternalInput")
    w_d = nc.dram_tensor("w", [2, D, D], F32, kind="ExternalInput")
    out_d = nc.dram_tensor("out", [n_rows, D], F32, kind="ExternalOutput")

    with tile.TileContext(nc) as tc:
        with tc.tile_pool(name="const", bufs=1) as constp, \
             tc.tile_pool(name="big", bufs=1) as bigp, \
             tc.tile_pool(name="xin", bufs=6) as xinp, \
             tc.tile_pool(name="pbuf", bufs=4) as pbufp, \
             tc.tile_pool(name="sm", bufs=4) as smp, \
             tc.tile_pool(name="osb", bufs=2) as osbp, \
             tc.tile_pool(name="ofin", bufs=2) as ofinp, \
             tc.tile_pool(name="est", bufs=3, space="PSUM") as estp, \
             tc.tile_pool(name="sq", bufs=2, space="PSUM") as sqp, \
             tc.tile_pool(name="pv", bufs=1, space="PSUM") as pvp:

            mT_t = constp.tile([D, D], F32R, tag="mT")   # (Wq' Wk^T)[d,e]
            wv_t = constp.tile([D, D], F32R, tag="wv")
            w_f = constp.tile([D, 2, D], F32, tag="w_f")
            nc.sync.dma_start(
                out=w_f[:],
                in_=w_d.ap().rearrange("w (p) d -> p w d", p=D))
            nc.scalar.copy(mT_t[:],
                           w_f[:, 0:1, :].rearrange("p a b -> p (a b)"))
            nc.scalar.copy(wv_t[:],
                           w_f[:, 1:2, :].rearrange("p a b -> p (a b)"))
            id_f = constp.tile([D, D], F32, tag="id_f")
            _make_identity(nc, id_f[:])

            rep_ctx = tc.For_i(0, m_repeat, 1) if m_repeat else \
                contextlib.nullcontext()
            rep_ctx.__enter__()

            for w in range(12):
                pwu = estp.tile([D, 512], F32, tag="est")
                nc.tensor.transpose(pwu[:, 0:D], id_f[:], id_f[:])

            xT = bigp.tile([D, n_tok], F32R, tag="xT")
            vTb = bigp.tile([D, n_tok], F16, tag="vTb")
            v_t = bigp.tile([D, tch, D], F16, tag="v_t")      # V tiles [d,j,t]
            pTa = bigp.tile([D, tch, 4, D], F16, tag="pTa")   # P^T blocks 0-3
            pTb = bigp.tile([D, tch, 4, D], F16, tag="pTb")   # P^T blocks 4-7
            l4all = bigp.tile([D, nb, nqt], F32, tag="l4all")
            l8 = bigp.tile([D, nb], F32, tag="l8")
            lrec = bigp.tile([D, nb], F32, tag="lrec")
            negms = {}

            def v_proj():
                for c in range(n_tok // 1024):
                    psq = sqp.tile([D, 1024], F32, tag="sq")
                    for h in range(2):
                        c0 = c * 1024 + h * 512
                        nc.tensor.matmul(psq[:, h * 512:(h + 1) * 512],
                                         lhsT=wv_t[:], rhs=xT[:, c0:c0 + 512],
                                         start=True, stop=True)
                    if c % 2 == 0:
                        nc.scalar.copy(vTb[:, c * 1024:(c + 1) * 1024],
                                       psq[:])
                    else:
                        nc.vector.tensor_copy(
                            vTb[:, c * 1024:(c + 1) * 1024], psq[:])
                nc.sync.dma_start_transpose(v_t[:], vTb[:])

            def est_mm(s, c):
                pe = estp.tile([D, 512], F32, tag="est")
                nc.tensor.matmul(pe[:], lhsT=gbs[s][:],
                                 rhs=xT[:, c * 512:(c + 1) * 512],
                                 start=True, stop=True)
                m8 = negms[s]["m8"]
                nc.vector.tensor_reduce(m8[:, c:c + 1], pe[:],
                                        axis=AX.X, op=ALU.max, negate=True)

            gbs = {}

            def g_make(s):
                gp = estp.tile([D, 512], F32, tag="est")
                nc.tensor.matmul(gp[:, 0:D], lhsT=mT_t[:],
                                 rhs=xT[:, s * 128:(s + 1) * 128],
                                 start=True, stop=True)
                gb = smp.tile([128, D], F32R, tag="gb")
                nc.scalar.copy(gb[:], gp[:, 0:D])
                gbs[s] = gb

            def est_begin(s):
                m8 = smp.tile([128, kch], F32, tag="m8")
                negm = smp.tile([128, 1], F32, tag="negm")
                negms[s] = {"m8": m8, "negm": negm}

            def est_end(s):
                nc.vector.tensor_reduce(negms[s]["negm"][:], negms[s]["m8"][:],
                                        axis=AX.X, op=ALU.min)

            def main_begin(s):
                ps = pbufp.tile([128, n_tok], F16, tag="p_s")
                negms[s]["ps"] = ps
                negms[s]["psq"] = None


            def main_mm(s, c):
                r0 = s * 128
                q, h = divmod(c, 2)
                if h == 0:
                    psq = sqp.tile([D, 1024], F32, tag="sq")
                    negms[s]["psq"] = psq
                psq = negms[s]["psq"]
                nc.tensor.matmul(psq[:, h * 512:(h + 1) * 512],
                                 lhsT=gbs[s][:],
                                 rhs=xT[:, c * 512:(c + 1) * 512],
                                 start=True, stop=True)
                if h == 1:
                    nc.scalar.activation(
                        negms[s]["ps"][:, q * qw:(q + 1) * qw], psq[:],
                        ACTF.Exp, bias=negms[s]["negm"][:, 0:1],
                        accum_out=l4all[:, s:s + 1, q:q + 1]
                            .rearrange("p a b -> p (a b)"))

            def main_end(s):
                pt = pTa if s < 4 else pTb
                if s == nb - 1:
                    # split the last block's transpose so PV pair{3}'s first
                    # chunks can start after the first half lands
                    for hh in range(2):
                        nc.sync.dma_start_transpose(
                            pt[:, hh * 16:(hh + 1) * 16, s % 4:s % 4 + 1, :]
                                .rearrange("p a b c -> p a (b c)"),
                            negms[s]["ps"][:, hh * 2048:(hh + 1) * 2048])
                else:
                    nc.sync.dma_start_transpose(
                        pt[:, :, s % 4:s % 4 + 1, :]
                            .rearrange("p a b c -> p a (b c)"),
                        negms[s]["ps"][:])

            def pv_mm(g, j0, j1, ppv):
                pt = pTa if g == 0 else pTb
                for j in range(j0, j1):
                    nc.tensor.matmul(
                        ppv[:],
                        lhsT=v_t[:, j:j + 1, :].rearrange("p a b -> p (a b)"),
                        rhs=pt[:, j:j + 1, :, :]
                            .rearrange("p a b c -> p a (b c)"),
                        start=(j == 0), stop=(j == tch - 1))

            def pv_mm_pair(pair, ppv):
                # block pair -> 256-col slice of its half's OUT^T
                pt = pTa if pair < 2 else pTb
                pp = pair % 2
                for j in range(tch):
                    nc.tensor.matmul(
                        ppv[:, pp * 256:(pp + 1) * 256],
                        lhsT=v_t[:, j:j + 1, :].rearrange("p a b -> p (a b)"),
                        rhs=pt[:, j:j + 1, pp * 2:(pp + 1) * 2, :]
                            .rearrange("p a b c -> p a (b c)"),
                        start=(j == 0), stop=(j == tch - 1))

            def pv_post(g, ppv):
                osb = osbp.tile([D, 512], F16, tag="osb")
                nc.scalar.copy(osb[:], ppv[:])
                pout = osbp.tile([128, 4, D], F16, tag="pout")
                nc.sync.dma_start_transpose(pout[:], osb[:])
                ofin = ofinp.tile([128, 4, D], F32, tag="ofin")
                for si in range(4):
                    nc.vector.tensor_scalar(
                        out=ofin[:, si:si + 1, :].rearrange("p a b -> p (a b)"),
                        in0=pout[:, si:si + 1, :].rearrange("p a b -> p (a b)"),
                        scalar1=lrec[:, g * 4 + si:g * 4 + si + 1],
                        scalar2=None, op0=ALU.mult)
                nc.sync.dma_start(
                    out=out_d.ap()[g * 512:(g + 1) * 512, :]
                        .rearrange("(s p) d -> p s d", p=D),
                    in_=ofin[:])

            # ---- setup: per-512-chunk pipeline, est(0) fused in ----
            est_begin(0)
            for c in range(kch):
                c0 = c * 512
                half = (c % 2) * 512
                if c % 2 == 0:
                    psq = sqp.tile([D, 1024], F32, tag="sq")
                slab = xinp.tile([D, 512], F32, tag="xslab")
                nc.sync.dma_start(
                    out=slab[:].rearrange("p (j d) -> p j d", d=D),
                    in_=x_d.ap()[c0:c0 + 512, :]
                        .rearrange("(j p) d -> p j d", p=D))
                for jj in range(4):
                    nc.tensor.transpose(
                        psq[:, half + jj * D:half + (jj + 1) * D],
                        slab[:, jj * D:(jj + 1) * D], id_f[:])
                xcopy = nc.scalar.copy if c % 2 == 0 else \
                    (lambda o, i: nc.vector.tensor_copy(o, i))
                xcopy(xT[:, c0:c0 + 512], psq[:, half:half + 512])
                if c == 0:
                    g_make(0)
                    g_make(1)
                est_mm(0, c)

            # ---- pipeline ----
            est_end(0)
            v_proj()
            ppv0 = pvp.tile([D, 512], F32, tag="pv")
            for s in range(1, nb + 1):
                if s <= nb - 1:
                    est_begin(s)
                    if s + 1 <= nb - 1:
                        g_make(s + 1)
                main_begin(s - 1)
                if s == 1:
                    for c in range(kch):
                        main_mm(0, c)
                    for c in range(kch):
                        est_mm(1, c)
                else:
                    for c in range(kch):
                        main_mm(s - 1, c)
                        if s <= nb - 1:
                            est_mm(s, c)
                if s <= nb - 1:
                    est_end(s)
                main_end(s - 1)
                if s - 1 == 3:
                    nc.vector.tensor_reduce(l8[:, 0:4], l4all[:, 0:4, :],
                                            axis=AX.X, op=ALU.add)
                    nc.vector.reciprocal(lrec[:, 0:4], l8[:, 0:4])
                if s - 1 == 5:
                    pv_mm(0, 0, 12, ppv0)
                if s - 1 == 6:
                    pv_mm(0, 12, 24, ppv0)
                    ppv1 = pvp.tile([D, 512], F32, tag="pv")
                    pv_mm_pair(2, ppv1)
            pv_mm(0, 24, tch, ppv0)
            nc.vector.tensor_reduce(l8[:, 4:8], l4all[:, 4:8, :],
                                    axis=AX.X, op=ALU.add)
            nc.vector.reciprocal(lrec[:, 4:8], l8[:, 4:8])
            pv_post(0, ppv0)
            pv_mm_pair(3, ppv1)
            pv_post(1, ppv1)

            rep_ctx.__exit__(None, None, None)
    nc.compile()
    return nc


def _get_nc():
    if "nc" not in _NC_CACHE:
        _NC_CACHE["nc"] = _build_attn()
    return _NC_CACHE["nc"]


def make_in_maps(input_embeddings, w_query, w_key, w_value):
    input_embeddings = np.asarray(input_embeddings, dtype=np.float32)
    b_sz, s_src, d = input_embeddings.shape
    assert (b_sz, s_src, d) == (B_SZ, S_SRC, D), "kernel compiled for 2x4096x128"
    wq_s = (np.asarray(w_query, dtype=np.float64) /
            np.float64(np.sqrt(d)))
    wk = np.asarray(w_key, dtype=np.float64)
    mT = (wq_s @ wk.T).astype(np.float32)      # (Wq' Wk^T)[d,e]
    wv = np.asarray(w_value, dtype=np.float32)
    wcat = np.ascontiguousarray(np.stack([mT, wv], axis=0))
    shards_per_b = N_CORES // b_sz
    in_maps = []
    for c in range(N_CORES):
        b, s = divmod(c, shards_per_b)
        x = np.ascontiguousarray(
            np.roll(input_embeddings[b], -s * N_ROWS, axis=0))
        in_maps.append({"x": x, "w": wcat})
    return in_maps


def kernel(input_embeddings, token_attention_masks_source=None,
           token_attention_masks_target=None, encoder_output_embedding=None,
           w_query=None, w_key=None, w_value=None, **_unused):
    """Full inputs in, full output out. Runs on 8 NeuronCores (SPMD)."""
    in_maps = make_in_maps(input_embeddings, w_query, w_key, w_value)
    res = run_bass_kernel_spmd(_get_nc(), in_maps, list(range(N_CORES)))
    out = np.empty((B_SZ, S_SRC, D), np.float32)
    shards_per_b = N_CORES // B_SZ
    for c in range(N_CORES):
        b, s = divmod(c, shards_per_b)
        out[b, s * N_ROWS:(s + 1) * N_ROWS] = res.results[c]["out"]
    return out

